# revision 40
# baseline (speedup 1.0000x reference)
"""DeepSeek-style MLA decode attention (batch=8, 128 heads, cache 512) on 8 NeuronCores.

Sharding: tensor-parallel over heads (16 heads/core), fp8/bf16 on-device.
 - Wq_down is REPLICATED on every core in fp8 (x64 host prescale): each core
   computes the full cq = x @ Wq_down locally, so no collective sits on the
   q critical path (cross-core NEFF launch skew made an early AllGather
   stall every core ~50us).
 - Wkv_down stays sharded over input rows (partial c_kv); one small
   AllGather ships the transposed ckv partials. Its only consumer (v_new)
   runs after the first attention group, far off the critical path.
 - k_cache/v_cache and Wq_up stored fp8e4 (Wq_up also x64; the 1/(64*64) is
   folded into the softmax exp scale). Wo fp16; other weights bf16.
 - Phase A uses a masked-q layout: qTm block hb is [128,32] with only column
   hb%32 live, so the 8 score MMs of a super accumulate into one [32,512]
   PSUM tile whose rows are the real score rows; softmax EXP reads the PSUM
   group tile directly.
 - k_cache host-pretransposed to [super, d, keys]; v_cache to [super, k, (c,t,d)].
 - o_proj band-interleaved: Wo sharded by input rows (head axis) and split
   into 4 head-bands x 14 column chunks; band a's MMs run right after
   attention group a finishes, overlapping the next group's attention.
   Each band's [8,512] chunk partials DMA straight out; the host sums the
   4 bands x 8 cores during unshard (no device ReduceScatter).

Note: the reference's "new token" softmax is over a length-1 axis (== 1.0), so
k_new/Wk_up are dead and the new-token contribution is simply + v_new.
"""

import numpy as np
import ml_dtypes

import concourse.bass as bass
import concourse.mybir as mybir
import concourse.tile as tile
from concourse import bacc
from concourse import bass_utils
from concourse.masks import make_identity

NC_ = 8                      # cores
B = 8                        # batch
H = 128                      # total heads
HP = H // NC_                # 16 heads per core
D = 128                      # head dim
L = 512                      # cache len
HID = 7168
QL = 1536
KVL = 512
KVRP = HID // NC_            # 896 input rows of Wkv_down per core
NH = HP * D                  # 2048 per-core head cols
W_SCALE = 64.0               # host premultiplies Wq_down and Wq_up by this
SCALE = 1.0 / float(np.sqrt(D))
F32 = mybir.dt.float32
F16 = mybir.dt.float16
BF16 = mybir.dt.bfloat16
F8 = mybir.dt.float8e4
U8 = mybir.dt.uint8
BF = ml_dtypes.bfloat16
E4 = ml_dtypes.float8_e4m3fn

NCHUNK = HID // 512          # 14 o_proj column chunks
NBAND = 4                    # o_proj head bands (4 heads each)


def build_nc():
    nc = bacc.Bacc(
        "TRN2",
        target_bir_lowering=False,
        debug=False,
        enable_asserts=False,
        num_devices=NC_,
    )
    xt = nc.dram_tensor("xt", [128, 28 * 64], F8, kind="ExternalInput").ap()
    xbf = nc.dram_tensor("xbf", [128, 56 * B], BF16, kind="ExternalInput").ap()
    wqd = nc.dram_tensor("wqd", [14, 128, 4 * QL], F8, kind="ExternalInput").ap()
    wkvd = nc.dram_tensor("wkvd", [14, 128, 4 * KVL], BF16, kind="ExternalInput").ap()
    wq = nc.dram_tensor("wq", [3, 128, 4 * NH], F8, kind="ExternalInput").ap()
    wvup = nc.dram_tensor("wvup", [2, 128, 2 * NH], BF16, kind="ExternalInput").ap()
    kt = nc.dram_tensor("kt", [16, 128, 4096], F8, kind="ExternalInput").ap()
    v = nc.dram_tensor("v", [16, 128, 4096], F8, kind="ExternalInput").ap()
    wo = nc.dram_tensor(
        "wo", [NBAND * NCHUNK, 128, 4 * 512], F16, kind="ExternalInput"
    ).ap()
    o = nc.dram_tensor("o", [NBAND * B, HID], F32, kind="ExternalOutput").ap()

    with tile.TileContext(nc) as tc:
        with (
            tc.tile_pool(name="const", bufs=1) as constp,
            tc.tile_pool(name="sbuf", bufs=1) as sb,
            tc.tile_pool(name="wqdp", bufs=6) as wqdp,
            tc.tile_pool(name="wkvdp", bufs=4) as wkvdp,
            tc.tile_pool(name="wqp", bufs=2) as wqp,
            tc.tile_pool(name="wvp", bufs=2) as wvp,
            tc.tile_pool(name="ktp", bufs=4) as ktp,
            tc.tile_pool(name="vp", bufs=4) as vp,
            tc.tile_pool(name="wop", bufs=10) as wop,
            tc.tile_pool(name="psbank", bufs=6, space="PSUM") as psbank,
            tc.tile_pool(name="pstr", bufs=2, space="PSUM") as pstr,
            tc.tile_pool(name="dram", bufs=1, space="DRAM") as dram,
        ):
            ident = constp.tile([128, 128], F32)
            make_identity(nc, ident[:])
            id8 = ident[0:8, 0:8]
            # uint8 one-hot columns for CopyPredicated masks (must be int dtype)
            identu8 = constp.tile([128, 128], U8, tag="identu8")
            nc.vector.tensor_copy(identu8[:], ident[:])

            xt_sb = constp.tile([128, 28 * 64], F8, tag="xt")
            nc.sync.dma_start(out=xt_sb[:], in_=xt)
            xbf_sb = constp.tile([128, 56 * B], BF16, tag="xbf")
            nc.sync.dma_start(out=xbf_sb[:], in_=xbf)

            # ---------------- cq = x @ Wq_down (full, local) ----------------
            # fp8 DoubleRow: contraction 7168 = 28 chunks of (128 part x 2
            # planes); xt_sb chunk ch is [128, (i,b)], wqd tile holds 2
            # chunks of [128, (i, 1536)].
            DR = mybir.MatmulPerfMode.DoubleRow
            ps_cqs = [
                psbank.tile([32, 512], F32, tag="bank", name=f"ps_cq{j}")
                for j in range(3)
            ]
            wqd_last = None
            for t in range(14):
                wqd_t = wqdp.tile([128, 4 * QL], F8, tag="wqd")
                nc.sync.dma_start(out=wqd_t[:], in_=wqd[t])
                wqd_last = wqd_t
                wqd_r = wqd_t[:].rearrange("p (l i c) -> p l i c", l=2, i=2)
                for ll in range(2):
                    ch = 2 * t + ll
                    lhs = xt_sb[:, ch * 64:(ch + 1) * 64].rearrange(
                        "p (i b) -> p i b", i=2
                    )
                    for j in range(3):
                        nc.tensor.matmul(
                            ps_cqs[j][0:32, :],
                            lhs,
                            wqd_r[:, ll, :, j * 512:(j + 1) * 512],
                            start=(ch == 0), stop=(ch == 27),
                            perf_mode=DR,
                        )
            # --- DMA gating tier 2: hold kt/v prefetch until wqd landed ---
            for _ in range(4):
                g_t = ktp.tile([128, 4096], F8, tag="kt", name="gate_kt")
                nc.vector.tensor_copy(g_t[0:1, 0:1], wqd_last[0:1, 0:1])
            for _ in range(4):
                g_t = vp.tile([128, 4096], F8, tag="v", name="gate_v")
                nc.vector.tensor_copy(g_t[0:1, 0:1], wqd_last[0:1, 0:1])

            cq_s = sb.tile([8, QL], F32, tag="cqs")
            for j in range(3):
                nc.vector.tensor_copy(
                    cq_s[:, j * 512:(j + 1) * 512], ps_cqs[j][0:8, :]
                )
            ps_cqT = pstr.tile([128, 96], F32, tag="tr")
            for r in range(12):
                nc.tensor.transpose(
                    ps_cqT[0:128, r * 8:(r + 1) * 8],
                    cq_s[:, r * 128:(r + 1) * 128],
                    id8,
                )
            # cq in fp8 at natural scale (cq_s holds 64x); padded to 32
            # cols per DoubleRow plane for the dual-fp8 ldweights restriction
            cqT = sb.tile([128, 6 * 64], F8, tag="cqT")
            nc.vector.memset(cqT[:], 0.0)
            for k in range(6):
                for i in range(2):
                    c = 2 * k + i
                    nc.scalar.activation(
                        cqT[:, k * 64 + i * 32:k * 64 + i * 32 + 8],
                        ps_cqT[:, c * 8:(c + 1) * 8],
                        mybir.ActivationFunctionType.Copy, scale=1.0 / W_SCALE,
                    )

            # ---------------- q = cq @ Wq_up_c  (8, 2048) ----------------
            # fp8 DoubleRow: contraction 1536 = 6 chunks of (128 x 2)
            qstage = sb.tile([8, NH], F32, tag="qstage")
            ps_qs = [
                psbank.tile([32, 512], F32, tag="bank", name=f"ps_q{n}")
                for n in range(4)
            ]
            for t in range(3):
                wq_t = wqp.tile([128, 4 * NH], F8, tag="wq")
                nc.sync.dma_start(out=wq_t[:], in_=wq[t])
                wq_r = wq_t[:].rearrange("p (l i c) -> p l i c", l=2, i=2)
                for ll in range(2):
                    k = 2 * t + ll
                    lhs = cqT[:, k * 64:(k + 1) * 64].rearrange(
                        "p (i b) -> p i b", i=2
                    )
                    for n in range(4):
                        nc.tensor.matmul(
                            ps_qs[n][0:32, :],
                            lhs,
                            wq_r[:, ll, :, n * 512:(n + 1) * 512],
                            start=(k == 0), stop=(k == 5),
                            perf_mode=DR,
                        )
            for n in range(4):
                nc.vector.tensor_copy(
                    qstage[:, n * 512:(n + 1) * 512], ps_qs[n][0:8, :]
                )

            # qT [128 d, 128 hb] fp8 at natural scale via 16 transposes
            ps_qT = pstr.tile([128, 128], F32, tag="tr", name="ps_qT")
            for h in range(HP):
                nc.tensor.transpose(
                    ps_qT[0:128, h * 8:(h + 1) * 8],
                    qstage[:, h * D:(h + 1) * D],
                    id8,
                )
            qT8 = sb.tile([128, 128], F8, tag="qT8")
            nc.scalar.activation(
                qT8[:], ps_qT[:],
                mybir.ActivationFunctionType.Copy, scale=1.0 / W_SCALE,
            )

            # masked q: qTm block hb = [128, 32], only column hb%32 live
            qTm = sb.tile([128, 128 * 32], F8, tag="qTm")
            nc.vector.memset(qTm[:], 0.0)

            # wvup DMAs issued early (prefetch); v_new compute deferred to
            # right after attention group 0 completes
            wvup_ts = []
            for j in range(2):
                wv_t = wvp.tile([128, 2 * NH], BF16, tag="wv", name=f"wvup{j}")
                nc.sync.dma_start(out=wv_t[:], in_=wvup[j])
                wvup_ts.append(wv_t)

            # ---------------- attention + banded o_proj pipeline ----------
            # group a = hb 32a..32a+32 (heads 4a..4a+3, all batches).
            # Phase A accumulates the group's 32 score rows into one base-0
            # [32,512] PSUM tile via the masked-q layout; per-group softmax
            # (scalar+vector only); phase B uses [32]-wide probsT column
            # slices; band a of o_proj (4 heads x 14 chunks) streams after
            # group a completes.
            #
            # The engines execute their instruction streams IN ORDER, so the
            # emission order below software-pipelines the PE stream: group
            # a+1's score MMs are emitted BEFORE group a's softmax-dependent
            # probs transposes, hiding the scalar/vector softmax latency
            # behind a full A-phase of PE work.
            probsT = sb.tile([128, 512], F8, tag="probsT")
            attnT = sb.tile([128, 128], F16, tag="attnT")
            id32 = ident[0:32, 0:32]
            vnewT = sb.tile([128, 128], F32, tag="vnewT")
            ps_gs = [None] * 4
            probsn = [None] * 4
            attn_as = [None] * 4

            def emit_A(a):
                ps_g = psbank.tile([32, 512], F32, tag="bank", name=f"ps_g{a}")
                ps_gs[a] = ps_g
                for s in range(4 * a, 4 * a + 4):
                    kt_t = ktp.tile([128, 4096], F8, tag="kt")
                    nc.sync.dma_start(out=kt_t[:], in_=kt[s])
                    for u in range(8):
                        hb = 8 * s + u
                        nc.vector.tensor_copy(
                            qTm[:, hb * 32 + (hb % 32):hb * 32 + (hb % 32) + 1],
                            qT8[:, hb:hb + 1],
                        )
                        nc.tensor.matmul(
                            ps_g[0:32, :],
                            qTm[:, hb * 32:(hb + 1) * 32],
                            kt_t[:, u * 512:(u + 1) * 512],
                            start=(s % 4 == 0 and u == 0),
                            stop=(s % 4 == 3 and u == 7),
                        )

            probs = [None] * 4
            denoms = [None] * 4

            def emit_exp(a):
                # scalar engine only: exp + row-sum of the group's scores
                probs_a = sb.tile([32, 512], F32, tag=f"probs{a % 2}")
                denom_a = sb.tile([32, 1], F32, tag=f"denom{a}")
                nc.scalar.activation(
                    probs_a[:], ps_gs[a][0:32, :],
                    mybir.ActivationFunctionType.Exp,
                    scale=SCALE, accum_out=denom_a[:],
                )
                probs[a] = probs_a
                denoms[a] = denom_a

            def emit_norm(a):
                # vector engine: normalize by the row sums
                recip_a = sb.tile([32, 1], F32, tag=f"recip{a}")
                nc.vector.reciprocal(recip_a[:], denoms[a][:])
                probsn_a = sb.tile([32, 512], F32, tag=f"probsn{a % 2}")
                nc.vector.tensor_scalar_mul(probsn_a[:], probs[a][:], recip_a[:])
                probsn[a] = probsn_a

            def emit_prT(a):
                # probsT_dr [64, (i, cc*128+hb)] fp8 = 16x normalized probs
                pa = 32 * a
                for cc in range(4):
                    ps_pT = pstr.tile([128, 32], F32, tag="tr")
                    nc.tensor.transpose(
                        ps_pT[:], probsn[a][0:32, cc * 128:(cc + 1) * 128], id32
                    )
                    nc.scalar.activation(
                        probsT[:, cc * 128 + pa:cc * 128 + pa + 32], ps_pT[:],
                        mybir.ActivationFunctionType.Copy, scale=16.0,
                    )

            def emit_B(a):
                pa = 32 * a
                attn_a = sb.tile([32, 128], F32, tag=f"attn{a}")
                attn_as[a] = attn_a
                pr_r = probsT[:].rearrange("p (pi i h) -> p pi i h", pi=2, i=2)
                for s in range(4 * a, 4 * a + 4):
                    v_t = vp.tile([128, 4096], F8, tag="v")
                    nc.sync.dma_start(out=v_t[:], in_=v[s])
                    for gg in range(2):
                        g = 2 * s + gg
                        ps_a = psbank.tile(
                            [32, 512], F32, tag="bank", name=f"ps_b{g}"
                        )
                        for pi in range(2):
                            rhs = v_t[:, gg * 2048 + pi * 1024:
                                      gg * 2048 + (pi + 1) * 1024].rearrange(
                                "p (i c) -> p i c", i=2
                            )
                            nc.tensor.matmul(
                                ps_a[0:32, :],
                                pr_r[:, pi, :, pa:pa + 32],
                                rhs,
                                start=(pi == 0), stop=(pi == 1),
                                perf_mode=DR,
                            )
                        for u in range(4):
                            hb = 4 * g + u
                            j = hb % 32
                            nc.vector.copy_predicated(
                                attn_a[0:32, :],
                                identu8[0:32, j:j + 1].broadcast_to((32, 128)),
                                ps_a[0:32, u * 128:(u + 1) * 128],
                            )

            def emit_vnew():
                # full local ckv = x @ Wkv_down (replicated weight), then v_new
                ps_ckv = psbank.tile([8, 512], F32, tag="bank", name="ps_ckv")
                for t in range(14):
                    wkvd_t = wkvdp.tile([128, 4 * KVL], BF16, tag="wkvd")
                    nc.sync.dma_start(out=wkvd_t[:], in_=wkvd[t])
                    for ii in range(4):
                        i = 4 * t + ii
                        nc.tensor.matmul(
                            ps_ckv[:8, :],
                            xbf_sb[:, i * B:(i + 1) * B],
                            wkvd_t[:, ii * KVL:(ii + 1) * KVL],
                            start=(i == 0), stop=(i == 55),
                        )
                cdkv = sb.tile([8, KVL], F32, tag="cdkv")
                nc.vector.tensor_copy(cdkv[:], ps_ckv[:8, :])
                ps_ckvT = pstr.tile([128, 32], F32, tag="tr")
                for j in range(4):
                    nc.tensor.transpose(
                        ps_ckvT[0:128, j * 8:(j + 1) * 8],
                        cdkv[:, j * 128:(j + 1) * 128],
                        id8,
                    )
                ckvT16 = sb.tile([128, 32], BF16, tag="ckvT16")
                nc.vector.tensor_copy(ckvT16[:], ps_ckvT[:, 0:32])
                vnew = sb.tile([8, NH], F32, tag="vnew")
                for n in range(4):
                    ps_v = psbank.tile([8, 512], F32, tag="bank")
                    for cc in range(4):
                        nc.tensor.matmul(
                            ps_v[:8, :],
                            ckvT16[:, cc * 8:(cc + 1) * 8],
                            wvup_ts[cc // 2][:, (cc % 2) * NH + n * 512:
                                             (cc % 2) * NH + (n + 1) * 512],
                            start=(cc == 0), stop=(cc == 3),
                        )
                    # x16 to match the 16x-scaled attention path
                    nc.scalar.activation(
                        vnew[:, n * 512:(n + 1) * 512], ps_v[:8, :],
                        mybir.ActivationFunctionType.Copy, scale=16.0,
                    )
                ps_vT = pstr.tile([128, 128], F32, tag="tr")
                for h in range(HP):
                    nc.tensor.transpose(
                        ps_vT[0:128, h * 8:(h + 1) * 8],
                        vnew[:, h * D:(h + 1) * D],
                        id8,
                    )
                nc.vector.tensor_copy(vnewT[:], ps_vT[:])

            def emit_aT(a):
                # band a of attnT: attn_a^T + vnewT slice
                pa = 32 * a
                ps_aT = pstr.tile([128, 32], F32, tag="tr", name=f"ps_aT{a}")
                nc.tensor.transpose(ps_aT[:], attn_as[a][:], id32)
                nc.vector.tensor_add(
                    attnT[:, pa:pa + 32], ps_aT[:], vnewT[:, pa:pa + 32]
                )

            def emit_C(a, lo=0, hi=NCHUNK):
                # band a of o_proj: chunks [lo, hi) x 4 accumulating MMs
                for c14 in range(lo, hi):
                    wo_t = wop.tile([128, 4 * 512], F16, tag="wo")
                    nc.sync.dma_start(out=wo_t[:], in_=wo[a * NCHUNK + c14])
                    ps_o = psbank.tile(
                        [8, 512], F32, tag="bank", name=f"ps_o{a}_{c14}"
                    )
                    for i in range(4):
                        nc.tensor.matmul(
                            ps_o[:8, :],
                            attnT[:, (4 * a + i) * 8:(4 * a + i + 1) * 8],
                            wo_t[:, i * 512:(i + 1) * 512],
                            start=(i == 0), stop=(i == 3),
                        )
                    ost = sb.tile([8, 512], F32, tag=f"ost{c14 % 2}")
                    # undo the 16x attention-path scaling
                    nc.scalar.activation(
                        ost[:], ps_o[:8, :],
                        mybir.ActivationFunctionType.Copy, scale=1.0 / 16.0,
                    )
                    nc.sync.dma_start(
                        out=o[a * B:(a + 1) * B, c14 * 512:(c14 + 1) * 512],
                        in_=ost[:],
                    )

            emit_A(0)
            emit_exp(0)
            emit_A(1)
            emit_exp(1)
            emit_norm(0)
            emit_prT(0)
            emit_B(0)
            emit_A(2)
            emit_exp(2)
            emit_norm(1)
            emit_prT(1)
            emit_vnew()
            emit_aT(0)
            emit_C(0, 0, 7)
            emit_B(1)
            emit_C(0, 7, 14)
            emit_A(3)
            emit_exp(3)
            emit_norm(2)
            emit_prT(2)
            emit_aT(1)
            emit_C(1, 0, 7)
            emit_B(2)
            emit_C(1, 7, 14)
            emit_norm(3)
            emit_prT(3)
            emit_aT(2)
            emit_C(2, 0, 7)
            emit_B(3)
            emit_C(2, 7, 14)
            emit_aT(3)
            emit_C(3)

    nc.compile()
    return nc


_NC_CACHE = None


def _get_nc():
    global _NC_CACHE
    if _NC_CACHE is None:
        _NC_CACHE = build_nc()
    return _NC_CACHE


def make_in_maps(x, k_cache, v_cache, Wq_down, Wq_up, Wkv_down, Wv_up, Wo):
    f16 = np.float16
    x2 = np.asarray(x, dtype=np.float32).reshape(B, HID).T  # [7168, 8]
    # DoubleRow plane layout, padded to 32 cols/plane:
    # [p, ch*64 + i*32 + b] = x2[ch*256 + i*128 + p, b] for b < 8, else 0
    xt_t = np.ascontiguousarray(
        np.pad(
            x2.reshape(28, 2, 128, B).transpose(2, 0, 1, 3),
            ((0, 0), (0, 0), (0, 0), (0, 24)),
        ).reshape(128, 28 * 64).astype(E4)
    )
    Wq_down = np.asarray(Wq_down, dtype=np.float32)
    Wq_up = np.asarray(Wq_up, dtype=np.float32)
    Wkv_down = np.asarray(Wkv_down, dtype=np.float32)
    Wv_up = np.asarray(Wv_up, dtype=np.float32)
    Wo = np.asarray(Wo, dtype=np.float32)
    k_cache = np.asarray(k_cache, dtype=np.float32)
    v_cache = np.asarray(v_cache, dtype=np.float32)

    # replicated full x (bf16) for the local ckv compute
    xbf_t = np.ascontiguousarray(
        x2.reshape(56, 128, B).transpose(1, 0, 2).reshape(128, 56 * B).astype(BF)
    )
    # replicated full Wkv_down bf16: [t][p, ii*512+c] = Wkvd[(4t+ii)*128+p, c]
    wkvd_r = np.ascontiguousarray(
        Wkv_down.reshape(14, 4, 128, KVL).transpose(0, 2, 1, 3)
        .reshape(14, 128, 4 * KVL).astype(BF)
    )
    # replicated: Wq_down x64 in fp8, DoubleRow planes:
    # [t][p, ll*3072 + i*1536 + c] = 64*Wqd[(2t+ll)*256 + i*128 + p, c]
    wqd_r = np.ascontiguousarray(
        (Wq_down * W_SCALE)
        .reshape(14, 2, 2, 128, QL).transpose(0, 3, 1, 2, 4)
        .reshape(14, 128, 4 * QL).astype(E4)
    )

    in_maps = []
    for c in range(NC_):
        hs = slice(c * HP, (c + 1) * HP)
        wq_c = (
            (Wq_up[:, c * NH:(c + 1) * NH] * W_SCALE)
            .reshape(3, 4, 128, NH).transpose(0, 2, 1, 3)
            .reshape(3, 128, 4 * NH).astype(E4)
        )
        wvup_c = (
            Wv_up[:, c * NH:(c + 1) * NH]
            .reshape(2, 2, 128, NH).transpose(0, 2, 1, 3).reshape(2, 128, 2 * NH)
            .astype(BF)
        )
        kt_c = (
            k_cache[:, hs]
            .transpose(1, 0, 3, 2)          # (16, 8, 128, 512) [h, b, d, k]
            .reshape(32, 4, 128, 512)       # [g, t, d, k]
            .transpose(0, 2, 1, 3)          # [g, d, t, k]
            .reshape(16, 2, 128, 2048)      # [s, g2, d, tk]
            .transpose(0, 2, 1, 3)
            .reshape(16, 128, 4096)
            .astype(E4)
        )
        v_c = (
            v_cache[:, hs]
            .transpose(1, 0, 2, 3)          # (16, 8, 512, 128) [h, b, l, d]
            .reshape(32, 4, 4, 128, 128)    # [g, t, c, k, d]
            .transpose(0, 3, 2, 1, 4)       # [g, k, c, t, d]
            .reshape(16, 2, 128, 2048)
            .transpose(0, 2, 1, 3)
            .reshape(16, 128, 4096)
            .astype(E4)
        )
        wo_shard = Wo[c * NH:(c + 1) * NH, :]  # [2048, 7168]
        # [band a, chunk c] tiles: [128, 4 heads * 512 cols]
        wo_c = (
            wo_shard
            .reshape(4, 4, 128, NCHUNK, 512)   # [a, i, p, c, j]
            .transpose(0, 3, 2, 1, 4)          # [a, c, p, i, j]
            .reshape(NBAND * NCHUNK, 128, 4 * 512)
            .astype(f16)
        )
        in_maps.append(
            {
                "xt": xt_t,
                "xbf": xbf_t,
                "wqd": wqd_r,
                "wkvd": wkvd_r,
                "wq": np.ascontiguousarray(wq_c),
                "wvup": np.ascontiguousarray(wvup_c),
                "kt": np.ascontiguousarray(kt_c),
                "v": np.ascontiguousarray(v_c),
                "wo": np.ascontiguousarray(wo_c),
            }
        )
    return in_maps


def gather_output(res):
    acc = np.zeros((B, HID), dtype=np.float32)
    for c in range(NC_):
        acc += res.results[c]["o"].reshape(NBAND, B, HID).sum(axis=0)
    return np.ascontiguousarray(acc.reshape(B, 1, HID))


def kernel(x, k_cache, v_cache, Wq_down, Wq_up, Wkv_down, Wk_up, Wv_up, Wo, **_):
    in_maps = make_in_maps(
        np.asarray(x), np.asarray(k_cache), np.asarray(v_cache),
        np.asarray(Wq_down), np.asarray(Wq_up),
        np.asarray(Wkv_down), np.asarray(Wv_up), np.asarray(Wo),
    )
    nc = _get_nc()
    res = bass_utils.run_bass_kernel_spmd(nc, in_maps, core_ids=list(range(NC_)))
    return gather_output(res)


# revision 41
# speedup vs baseline: 1.0767x; 1.0767x over previous
"""DeepSeek-style MLA decode attention (batch=8, 128 heads, cache 512) on 8 NeuronCores.

Sharding: tensor-parallel over heads (16 heads/core), fp8/bf16 on-device.
 - Wq_down is REPLICATED on every core in fp8 (x64 host prescale): each core
   computes the full cq = x @ Wq_down locally, so no collective sits on the
   q critical path (cross-core NEFF launch skew made an early AllGather
   stall every core ~50us).
 - Wkv_down stays sharded over input rows (partial c_kv); one small
   AllGather ships the transposed ckv partials. Its only consumer (v_new)
   runs after the first attention group, far off the critical path.
 - k_cache/v_cache and Wq_up stored fp8e4 (Wq_up also x64; the 1/(64*64) is
   folded into the softmax exp scale). Wo fp16; other weights bf16.
 - Phase A uses a masked-q layout: qTm block hb is [128,32] with only column
   hb%32 live, so the 8 score MMs of a super accumulate into one [32,512]
   PSUM tile whose rows are the real score rows; softmax EXP reads the PSUM
   group tile directly.
 - k_cache host-pretransposed to [super, d, keys]; v_cache to [super, k, (c,t,d)].
 - o_proj band-interleaved: Wo sharded by input rows (head axis) and split
   into 4 head-bands x 14 column chunks; band a's MMs run right after
   attention group a finishes, overlapping the next group's attention.
   Each band's [8,512] chunk partials DMA straight out; the host sums the
   4 bands x 8 cores during unshard (no device ReduceScatter).

Note: the reference's "new token" softmax is over a length-1 axis (== 1.0), so
k_new/Wk_up are dead and the new-token contribution is simply + v_new.
"""

import numpy as np
import ml_dtypes

import concourse.bass as bass
import concourse.mybir as mybir
import concourse.tile as tile
from concourse import bacc
from concourse import bass_utils
from concourse.masks import make_identity

NC_ = 8                      # cores
B = 8                        # batch
H = 128                      # total heads
HP = H // NC_                # 16 heads per core
D = 128                      # head dim
L = 512                      # cache len
HID = 7168
QL = 1536
KVL = 512
KVRP = HID // NC_            # 896 input rows of Wkv_down per core
NH = HP * D                  # 2048 per-core head cols
W_SCALE = 64.0               # host premultiplies Wq_down and Wq_up by this
SCALE = 1.0 / float(np.sqrt(D))
F32 = mybir.dt.float32
F16 = mybir.dt.float16
BF16 = mybir.dt.bfloat16
F8 = mybir.dt.float8e4
U8 = mybir.dt.uint8
BF = ml_dtypes.bfloat16
E4 = ml_dtypes.float8_e4m3fn

NCHUNK = HID // 512          # 14 o_proj column chunks
NBAND = 4                    # o_proj head bands (4 heads each)


def build_nc():
    nc = bacc.Bacc(
        "TRN2",
        target_bir_lowering=False,
        debug=False,
        enable_asserts=False,
        num_devices=NC_,
    )
    xt = nc.dram_tensor("xt", [128, 28 * 64], F8, kind="ExternalInput").ap()
    xbf = nc.dram_tensor("xbf", [128, 56 * B], BF16, kind="ExternalInput").ap()
    wqd = nc.dram_tensor("wqd", [14, 128, 4 * QL], F8, kind="ExternalInput").ap()
    wkvd = nc.dram_tensor("wkvd", [14, 128, 4 * KVL], BF16, kind="ExternalInput").ap()
    wq = nc.dram_tensor("wq", [3, 128, 4 * NH], F8, kind="ExternalInput").ap()
    wvup = nc.dram_tensor("wvup", [2, 128, 2 * NH], BF16, kind="ExternalInput").ap()
    kt = nc.dram_tensor("kt", [16, 128, 4096], F8, kind="ExternalInput").ap()
    v = nc.dram_tensor("v", [16, 128, 4096], F8, kind="ExternalInput").ap()
    wo = nc.dram_tensor(
        "wo", [NBAND * NCHUNK, 128, 4 * 512], F16, kind="ExternalInput"
    ).ap()
    o = nc.dram_tensor("o", [NBAND * B, HID], F32, kind="ExternalOutput").ap()

    with tile.TileContext(nc) as tc:
        with (
            tc.tile_pool(name="const", bufs=1) as constp,
            tc.tile_pool(name="sbuf", bufs=1) as sb,
            tc.tile_pool(name="wqdp", bufs=6) as wqdp,
            tc.tile_pool(name="wkvdp", bufs=4) as wkvdp,
            tc.tile_pool(name="wqp", bufs=2) as wqp,
            tc.tile_pool(name="wvp", bufs=2) as wvp,
            tc.tile_pool(name="ktp", bufs=4) as ktp,
            tc.tile_pool(name="vp", bufs=4) as vp,
            tc.tile_pool(name="wop", bufs=10) as wop,
            tc.tile_pool(name="psbank", bufs=6, space="PSUM") as psbank,
            tc.tile_pool(name="pstr", bufs=2, space="PSUM") as pstr,
            tc.tile_pool(name="dram", bufs=1, space="DRAM") as dram,
        ):
            ident = constp.tile([128, 128], F32)
            make_identity(nc, ident[:])
            id8 = ident[0:8, 0:8]
            # uint8 one-hot columns for CopyPredicated masks (must be int dtype)
            identu8 = constp.tile([128, 128], U8, tag="identu8")
            nc.vector.tensor_copy(identu8[:], ident[:])

            xt_sb = constp.tile([128, 28 * 64], F8, tag="xt")
            nc.sync.dma_start(out=xt_sb[:], in_=xt)
            xbf_sb = constp.tile([128, 56 * B], BF16, tag="xbf")
            nc.sync.dma_start(out=xbf_sb[:], in_=xbf)

            # ---------------- cq = x @ Wq_down (full, local) ----------------
            # fp8 DoubleRow: contraction 7168 = 28 chunks of (128 part x 2
            # planes); xt_sb chunk ch is [128, (i,b)], wqd tile holds 2
            # chunks of [128, (i, 1536)].
            DR = mybir.MatmulPerfMode.DoubleRow
            ps_cqs = [
                psbank.tile([32, 512], F32, tag="bank", name=f"ps_cq{j}")
                for j in range(3)
            ]
            wqd_last = None
            for t in range(14):
                wqd_t = wqdp.tile([128, 4 * QL], F8, tag="wqd")
                nc.sync.dma_start(out=wqd_t[:], in_=wqd[t])
                wqd_last = wqd_t
                wqd_r = wqd_t[:].rearrange("p (l i c) -> p l i c", l=2, i=2)
                for ll in range(2):
                    ch = 2 * t + ll
                    lhs = xt_sb[:, ch * 64:(ch + 1) * 64].rearrange(
                        "p (i b) -> p i b", i=2
                    )
                    for j in range(3):
                        nc.tensor.matmul(
                            ps_cqs[j][0:32, :],
                            lhs,
                            wqd_r[:, ll, :, j * 512:(j + 1) * 512],
                            start=(ch == 0), stop=(ch == 27),
                            perf_mode=DR,
                        )
            # --- DMA gating tier 2: hold kt/v prefetch until wqd landed ---
            for _ in range(4):
                g_t = ktp.tile([128, 4096], F8, tag="kt", name="gate_kt")
                nc.vector.tensor_copy(g_t[0:1, 0:1], wqd_last[0:1, 0:1])
            for _ in range(4):
                g_t = vp.tile([128, 4096], F8, tag="v", name="gate_v")
                nc.vector.tensor_copy(g_t[0:1, 0:1], wqd_last[0:1, 0:1])

            cq_s = sb.tile([8, QL], F32, tag="cqs")
            for j in range(3):
                nc.vector.tensor_copy(
                    cq_s[:, j * 512:(j + 1) * 512], ps_cqs[j][0:8, :]
                )
            ps_cqT = pstr.tile([128, 96], F32, tag="tr")
            for r in range(12):
                nc.tensor.transpose(
                    ps_cqT[0:128, r * 8:(r + 1) * 8],
                    cq_s[:, r * 128:(r + 1) * 128],
                    id8,
                )
            # cq in fp8 at natural scale (cq_s holds 64x); padded to 32
            # cols per DoubleRow plane for the dual-fp8 ldweights restriction
            cqT = sb.tile([128, 6 * 64], F8, tag="cqT")
            nc.vector.memset(cqT[:], 0.0)
            for k in range(6):
                for i in range(2):
                    c = 2 * k + i
                    nc.scalar.activation(
                        cqT[:, k * 64 + i * 32:k * 64 + i * 32 + 8],
                        ps_cqT[:, c * 8:(c + 1) * 8],
                        mybir.ActivationFunctionType.Copy, scale=1.0 / W_SCALE,
                    )

            # ---------------- q = cq @ Wq_up_c  (8, 2048) ----------------
            # fp8 DoubleRow: contraction 1536 = 6 chunks of (128 x 2)
            qstage = sb.tile([8, NH], F32, tag="qstage")
            ps_qs = [
                psbank.tile([32, 512], F32, tag="bank", name=f"ps_q{n}")
                for n in range(4)
            ]
            for t in range(3):
                wq_t = wqp.tile([128, 4 * NH], F8, tag="wq")
                nc.sync.dma_start(out=wq_t[:], in_=wq[t])
                wq_r = wq_t[:].rearrange("p (l i c) -> p l i c", l=2, i=2)
                for ll in range(2):
                    k = 2 * t + ll
                    lhs = cqT[:, k * 64:(k + 1) * 64].rearrange(
                        "p (i b) -> p i b", i=2
                    )
                    for n in range(4):
                        nc.tensor.matmul(
                            ps_qs[n][0:32, :],
                            lhs,
                            wq_r[:, ll, :, n * 512:(n + 1) * 512],
                            start=(k == 0), stop=(k == 5),
                            perf_mode=DR,
                        )
            for n in range(4):
                nc.vector.tensor_copy(
                    qstage[:, n * 512:(n + 1) * 512], ps_qs[n][0:8, :]
                )

            # qT [128 d, 128 hb] fp8 at natural scale via 16 transposes
            ps_qT = pstr.tile([128, 128], F32, tag="tr", name="ps_qT")
            for h in range(HP):
                nc.tensor.transpose(
                    ps_qT[0:128, h * 8:(h + 1) * 8],
                    qstage[:, h * D:(h + 1) * D],
                    id8,
                )
            qT8 = sb.tile([128, 128], F8, tag="qT8")
            nc.scalar.activation(
                qT8[:], ps_qT[:],
                mybir.ActivationFunctionType.Copy, scale=1.0 / W_SCALE,
            )

            # masked q: qTm block hb = [128, 32], only column hb%32 live
            qTm = sb.tile([128, 128 * 32], F8, tag="qTm")
            nc.vector.memset(qTm[:], 0.0)

            # wvup DMAs issued early (prefetch); v_new compute deferred to
            # right after attention group 0 completes
            wvup_ts = []
            for j in range(2):
                wv_t = wvp.tile([128, 2 * NH], BF16, tag="wv", name=f"wvup{j}")
                nc.sync.dma_start(out=wv_t[:], in_=wvup[j])
                wvup_ts.append(wv_t)

            # ---------------- attention + banded o_proj pipeline ----------
            # group a = hb 32a..32a+32 (heads 4a..4a+3, all batches).
            # Phase A accumulates the group's 32 score rows into one base-0
            # [32,512] PSUM tile via the masked-q layout; per-group softmax
            # (scalar+vector only); phase B uses [32]-wide probsT column
            # slices; band a of o_proj (4 heads x 14 chunks) streams after
            # group a completes.
            #
            # The engines execute their instruction streams IN ORDER, so the
            # emission order below software-pipelines the PE stream: group
            # a+1's score MMs are emitted BEFORE group a's softmax-dependent
            # probs transposes, hiding the scalar/vector softmax latency
            # behind a full A-phase of PE work.
            probsT = sb.tile([128, 512], F8, tag="probsT")
            attnT = sb.tile([128, 128], F16, tag="attnT")
            id32 = ident[0:32, 0:32]
            vnewT = sb.tile([128, 128], F32, tag="vnewT")
            ps_gs = [None] * 4
            probsn = [None] * 4
            attn_as = [None] * 4

            def emit_A(a):
                ps_g = psbank.tile([32, 512], F32, tag="bank", name=f"ps_g{a}")
                ps_gs[a] = ps_g
                for s in range(4 * a, 4 * a + 4):
                    kt_t = ktp.tile([128, 4096], F8, tag="kt")
                    nc.sync.dma_start(out=kt_t[:], in_=kt[s])
                    for u in range(8):
                        hb = 8 * s + u
                        nc.vector.tensor_copy(
                            qTm[:, hb * 32 + (hb % 32):hb * 32 + (hb % 32) + 1],
                            qT8[:, hb:hb + 1],
                        )
                        nc.tensor.matmul(
                            ps_g[0:32, :],
                            qTm[:, hb * 32:(hb + 1) * 32],
                            kt_t[:, u * 512:(u + 1) * 512],
                            start=(s % 4 == 0 and u == 0),
                            stop=(s % 4 == 3 and u == 7),
                        )

            probs = [None] * 4
            denoms = [None] * 4

            def emit_exp(a):
                # scalar engine only: exp + row-sum of the group's scores
                probs_a = sb.tile([32, 512], F32, tag=f"probs{a % 2}")
                denom_a = sb.tile([32, 1], F32, tag=f"denom{a}")
                nc.scalar.activation(
                    probs_a[:], ps_gs[a][0:32, :],
                    mybir.ActivationFunctionType.Exp,
                    scale=SCALE, accum_out=denom_a[:],
                )
                probs[a] = probs_a
                denoms[a] = denom_a

            def emit_norm(a):
                # vector engine: normalize by the row sums
                recip_a = sb.tile([32, 1], F32, tag=f"recip{a}")
                nc.vector.reciprocal(recip_a[:], denoms[a][:])
                probsn_a = sb.tile([32, 512], F32, tag=f"probsn{a % 2}")
                nc.vector.tensor_scalar_mul(probsn_a[:], probs[a][:], recip_a[:])
                probsn[a] = probsn_a

            def emit_prT(a):
                # probsT_dr [64, (i, cc*128+hb)] fp8 = 16x normalized probs
                pa = 32 * a
                for cc in range(4):
                    ps_pT = pstr.tile([128, 32], F32, tag="tr")
                    nc.tensor.transpose(
                        ps_pT[:], probsn[a][0:32, cc * 128:(cc + 1) * 128], id32
                    )
                    nc.scalar.activation(
                        probsT[:, cc * 128 + pa:cc * 128 + pa + 32], ps_pT[:],
                        mybir.ActivationFunctionType.Copy, scale=16.0,
                    )

            def emit_B(a):
                pa = 32 * a
                attn_a = sb.tile([32, 128], F32, tag=f"attn{a}")
                attn_as[a] = attn_a
                pr_r = probsT[:].rearrange("p (pi i h) -> p pi i h", pi=2, i=2)
                for s in range(4 * a, 4 * a + 4):
                    v_t = vp.tile([128, 4096], F8, tag="v")
                    nc.sync.dma_start(out=v_t[:], in_=v[s])
                    for gg in range(2):
                        g = 2 * s + gg
                        ps_a = psbank.tile(
                            [32, 512], F32, tag="bank", name=f"ps_b{g}"
                        )
                        for pi in range(2):
                            rhs = v_t[:, gg * 2048 + pi * 1024:
                                      gg * 2048 + (pi + 1) * 1024].rearrange(
                                "p (i c) -> p i c", i=2
                            )
                            nc.tensor.matmul(
                                ps_a[0:32, :],
                                pr_r[:, pi, :, pa:pa + 32],
                                rhs,
                                start=(pi == 0), stop=(pi == 1),
                                perf_mode=DR,
                            )
                        for u in range(4):
                            hb = 4 * g + u
                            j = hb % 32
                            nc.vector.copy_predicated(
                                attn_a[0:32, :],
                                identu8[0:32, j:j + 1].broadcast_to((32, 128)),
                                ps_a[0:32, u * 128:(u + 1) * 128],
                            )

            def emit_vnew():
                # full local ckv = x @ Wkv_down (replicated weight), then v_new
                ps_ckv = psbank.tile([8, 512], F32, tag="bank", name="ps_ckv")
                for t in range(14):
                    wkvd_t = wkvdp.tile([128, 4 * KVL], BF16, tag="wkvd")
                    nc.sync.dma_start(out=wkvd_t[:], in_=wkvd[t])
                    for ii in range(4):
                        i = 4 * t + ii
                        nc.tensor.matmul(
                            ps_ckv[:8, :],
                            xbf_sb[:, i * B:(i + 1) * B],
                            wkvd_t[:, ii * KVL:(ii + 1) * KVL],
                            start=(i == 0), stop=(i == 55),
                        )
                cdkv = sb.tile([8, KVL], F32, tag="cdkv")
                nc.vector.tensor_copy(cdkv[:], ps_ckv[:8, :])
                ps_ckvT = pstr.tile([128, 32], F32, tag="tr")
                for j in range(4):
                    nc.tensor.transpose(
                        ps_ckvT[0:128, j * 8:(j + 1) * 8],
                        cdkv[:, j * 128:(j + 1) * 128],
                        id8,
                    )
                ckvT16 = sb.tile([128, 32], BF16, tag="ckvT16")
                nc.vector.tensor_copy(ckvT16[:], ps_ckvT[:, 0:32])
                vnew = sb.tile([8, NH], F32, tag="vnew")
                for n in range(4):
                    ps_v = psbank.tile([8, 512], F32, tag="bank")
                    for cc in range(4):
                        nc.tensor.matmul(
                            ps_v[:8, :],
                            ckvT16[:, cc * 8:(cc + 1) * 8],
                            wvup_ts[cc // 2][:, (cc % 2) * NH + n * 512:
                                             (cc % 2) * NH + (n + 1) * 512],
                            start=(cc == 0), stop=(cc == 3),
                        )
                    # x16 to match the 16x-scaled attention path
                    nc.scalar.activation(
                        vnew[:, n * 512:(n + 1) * 512], ps_v[:8, :],
                        mybir.ActivationFunctionType.Copy, scale=16.0,
                    )
                ps_vT = pstr.tile([128, 128], F32, tag="tr")
                for h in range(HP):
                    nc.tensor.transpose(
                        ps_vT[0:128, h * 8:(h + 1) * 8],
                        vnew[:, h * D:(h + 1) * D],
                        id8,
                    )
                nc.vector.tensor_copy(vnewT[:], ps_vT[:])

            def emit_aT(a):
                # band a of attnT: attn_a^T + vnewT slice
                pa = 32 * a
                ps_aT = pstr.tile([128, 32], F32, tag="tr", name=f"ps_aT{a}")
                nc.tensor.transpose(ps_aT[:], attn_as[a][:], id32)
                nc.vector.tensor_add(
                    attnT[:, pa:pa + 32], ps_aT[:], vnewT[:, pa:pa + 32]
                )

            def emit_C(a):
                # band a of o_proj: 14 chunks x 4 accumulating MMs
                for c14 in range(NCHUNK):
                    wo_t = wop.tile([128, 4 * 512], F16, tag="wo")
                    nc.sync.dma_start(out=wo_t[:], in_=wo[a * NCHUNK + c14])
                    ps_o = psbank.tile(
                        [8, 512], F32, tag="bank", name=f"ps_o{a}_{c14}"
                    )
                    for i in range(4):
                        nc.tensor.matmul(
                            ps_o[:8, :],
                            attnT[:, (4 * a + i) * 8:(4 * a + i + 1) * 8],
                            wo_t[:, i * 512:(i + 1) * 512],
                            start=(i == 0), stop=(i == 3),
                        )
                    ost = sb.tile([8, 512], F32, tag=f"ost{c14 % 2}")
                    # undo the 16x attention-path scaling
                    nc.scalar.activation(
                        ost[:], ps_o[:8, :],
                        mybir.ActivationFunctionType.Copy, scale=1.0 / 16.0,
                    )
                    nc.sync.dma_start(
                        out=o[a * B:(a + 1) * B, c14 * 512:(c14 + 1) * 512],
                        in_=ost[:],
                    )

            emit_A(0)
            emit_exp(0)
            emit_A(1)
            emit_exp(1)
            emit_norm(0)
            emit_prT(0)
            emit_B(0)
            emit_A(2)
            emit_exp(2)
            emit_norm(1)
            emit_prT(1)
            emit_vnew()
            emit_aT(0)
            emit_C(0)
            emit_B(1)
            emit_A(3)
            emit_exp(3)
            emit_norm(2)
            emit_prT(2)
            emit_aT(1)
            emit_C(1)
            emit_B(2)
            emit_norm(3)
            emit_prT(3)
            emit_aT(2)
            emit_C(2)
            emit_B(3)
            emit_aT(3)
            emit_C(3)

    nc.compile()
    return nc


_NC_CACHE = None


def _get_nc():
    global _NC_CACHE
    if _NC_CACHE is None:
        _NC_CACHE = build_nc()
    return _NC_CACHE


def make_in_maps(x, k_cache, v_cache, Wq_down, Wq_up, Wkv_down, Wv_up, Wo):
    f16 = np.float16
    x2 = np.asarray(x, dtype=np.float32).reshape(B, HID).T  # [7168, 8]
    # DoubleRow plane layout, padded to 32 cols/plane:
    # [p, ch*64 + i*32 + b] = x2[ch*256 + i*128 + p, b] for b < 8, else 0
    xt_t = np.ascontiguousarray(
        np.pad(
            x2.reshape(28, 2, 128, B).transpose(2, 0, 1, 3),
            ((0, 0), (0, 0), (0, 0), (0, 24)),
        ).reshape(128, 28 * 64).astype(E4)
    )
    Wq_down = np.asarray(Wq_down, dtype=np.float32)
    Wq_up = np.asarray(Wq_up, dtype=np.float32)
    Wkv_down = np.asarray(Wkv_down, dtype=np.float32)
    Wv_up = np.asarray(Wv_up, dtype=np.float32)
    Wo = np.asarray(Wo, dtype=np.float32)
    k_cache = np.asarray(k_cache, dtype=np.float32)
    v_cache = np.asarray(v_cache, dtype=np.float32)

    # replicated full x (bf16) for the local ckv compute
    xbf_t = np.ascontiguousarray(
        x2.reshape(56, 128, B).transpose(1, 0, 2).reshape(128, 56 * B).astype(BF)
    )
    # replicated full Wkv_down bf16: [t][p, ii*512+c] = Wkvd[(4t+ii)*128+p, c]
    wkvd_r = np.ascontiguousarray(
        Wkv_down.reshape(14, 4, 128, KVL).transpose(0, 2, 1, 3)
        .reshape(14, 128, 4 * KVL).astype(BF)
    )
    # replicated: Wq_down x64 in fp8, DoubleRow planes:
    # [t][p, ll*3072 + i*1536 + c] = 64*Wqd[(2t+ll)*256 + i*128 + p, c]
    wqd_r = np.ascontiguousarray(
        (Wq_down * W_SCALE)
        .reshape(14, 2, 2, 128, QL).transpose(0, 3, 1, 2, 4)
        .reshape(14, 128, 4 * QL).astype(E4)
    )

    in_maps = []
    for c in range(NC_):
        hs = slice(c * HP, (c + 1) * HP)
        wq_c = (
            (Wq_up[:, c * NH:(c + 1) * NH] * W_SCALE)
            .reshape(3, 4, 128, NH).transpose(0, 2, 1, 3)
            .reshape(3, 128, 4 * NH).astype(E4)
        )
        wvup_c = (
            Wv_up[:, c * NH:(c + 1) * NH]
            .reshape(2, 2, 128, NH).transpose(0, 2, 1, 3).reshape(2, 128, 2 * NH)
            .astype(BF)
        )
        kt_c = (
            k_cache[:, hs]
            .transpose(1, 0, 3, 2)          # (16, 8, 128, 512) [h, b, d, k]
            .reshape(32, 4, 128, 512)       # [g, t, d, k]
            .transpose(0, 2, 1, 3)          # [g, d, t, k]
            .reshape(16, 2, 128, 2048)      # [s, g2, d, tk]
            .transpose(0, 2, 1, 3)
            .reshape(16, 128, 4096)
            .astype(E4)
        )
        v_c = (
            v_cache[:, hs]
            .transpose(1, 0, 2, 3)          # (16, 8, 512, 128) [h, b, l, d]
            .reshape(32, 4, 4, 128, 128)    # [g, t, c, k, d]
            .transpose(0, 3, 2, 1, 4)       # [g, k, c, t, d]
            .reshape(16, 2, 128, 2048)
            .transpose(0, 2, 1, 3)
            .reshape(16, 128, 4096)
            .astype(E4)
        )
        wo_shard = Wo[c * NH:(c + 1) * NH, :]  # [2048, 7168]
        # [band a, chunk c] tiles: [128, 4 heads * 512 cols]
        wo_c = (
            wo_shard
            .reshape(4, 4, 128, NCHUNK, 512)   # [a, i, p, c, j]
            .transpose(0, 3, 2, 1, 4)          # [a, c, p, i, j]
            .reshape(NBAND * NCHUNK, 128, 4 * 512)
            .astype(f16)
        )
        in_maps.append(
            {
                "xt": xt_t,
                "xbf": xbf_t,
                "wqd": wqd_r,
                "wkvd": wkvd_r,
                "wq": np.ascontiguousarray(wq_c),
                "wvup": np.ascontiguousarray(wvup_c),
                "kt": np.ascontiguousarray(kt_c),
                "v": np.ascontiguousarray(v_c),
                "wo": np.ascontiguousarray(wo_c),
            }
        )
    return in_maps


def gather_output(res):
    acc = np.zeros((B, HID), dtype=np.float32)
    for c in range(NC_):
        acc += res.results[c]["o"].reshape(NBAND, B, HID).sum(axis=0)
    return np.ascontiguousarray(acc.reshape(B, 1, HID))


def kernel(x, k_cache, v_cache, Wq_down, Wq_up, Wkv_down, Wk_up, Wv_up, Wo, **_):
    in_maps = make_in_maps(
        np.asarray(x), np.asarray(k_cache), np.asarray(v_cache),
        np.asarray(Wq_down), np.asarray(Wq_up),
        np.asarray(Wkv_down), np.asarray(Wv_up), np.asarray(Wo),
    )
    nc = _get_nc()
    res = bass_utils.run_bass_kernel_spmd(nc, in_maps, core_ids=list(range(NC_)))
    return gather_output(res)
